# revision 23
# baseline (speedup 1.0000x reference)
"""BidLatte (linear-attention) Trainium2 kernel, 8-core SPMD.

Math (per batch b):
  K = X@Wk; Q = X@Wq; E = exp(K)*mask          (max-shift cancels exactly)
  Ksum = sum_t E;  KX = E^T @ X                (L x D state, avoids X@Wv)
  KXn = KX / Ksum; Kv = KXn @ Wv; Kv_bd = blockdiag_head(Kv)
  G = Kv_bd @ o_proj                           (o_proj folded into state)
  out = softmax_head(Q) @ G

Sharding: core 2i+j -> batch i, T-half j.

Schedule:
  A-K:   K-side state accumulation (DMA-bound).  X natural blocks stream
         on the sync HWDGE ring, X^T blocks on the gpsimd SWDGE ring
         (concurrent rings; HWDGE DMAs are FIFO per issuing engine) into
         a resident SBUF X^T tile reused by the Q side.  A tiny warm-up
         AllGather early in A-K absorbs the ~11us ncfw cold-start.
  RS:    ReduceScatter of the (128 x 1032) packed state -- each core of
         a pair keeps the 8-head half it needs for phase B.
  A-Q:   blocks 0-5 of the Q side (projection + head-softmax) off the
         resident X^T cover the RS.
  B:     per-core HALF of KXn@Wv + blockdiag + @o_proj (only half of
         Wv / o_proj is ever DMA'd), with dummy matmuls interleaved so
         the HAM clock gate stays released through the DVE-heavy chain.
  A-Q2:  blocks 6-7 of the Q side cover the small bf16 AllGather of G.
  C:     out = softmax(Q)^T^T @ G, stored as bf16 (host converts to
         f32) to halve the output-store traffic; out-DMAs alternate
         between the sync and scalar rings.

All matmuls bf16 (full PE rate); elementwise/exp/reductions and PSUM
accumulation fp32. Measured rel err vs fp32 reference ~5.7e-3.
"""
import numpy as np

_B, _T, _D, _L, _H = 4, 8192, 1024, 128, 16
NCORES = 8
TLOC = _T // 2  # tokens per core
BT = 512        # tokens per block
NBLK = TLOC // BT
NT = BT // 128  # t-tiles per block
DC = _D // 128  # d-chunks

_cache = {}


def _build():
    import concourse.bacc as bacc
    import concourse.mybir as mybir
    import concourse.tile as tile

    FP32 = mybir.dt.float32
    BF16 = mybir.dt.bfloat16
    EXP = mybir.ActivationFunctionType.Exp

    nc = bacc.Bacc("TRN2", target_bir_lowering=False, debug=False,
                   num_devices=NCORES)

    xs = nc.dram_tensor("xs", [TLOC, _D], BF16, kind="ExternalInput")
    xst = nc.dram_tensor("xst", [_D, TLOC], BF16, kind="ExternalInput")
    ms = nc.dram_tensor("ms", [128, TLOC // 128], FP32, kind="ExternalInput")
    wk = nc.dram_tensor("wk", [128, _D], BF16, kind="ExternalInput")
    wq = nc.dram_tensor("wq", [128, _D], BF16, kind="ExternalInput")
    wv = nc.dram_tensor("wv", [128, DC * 512], BF16, kind="ExternalInput")
    op = nc.dram_tensor("op", [128, 4 * _D], BF16, kind="ExternalInput")
    ident = nc.dram_tensor("ident", [128, 128], BF16, kind="ExternalInput")
    ph = nc.dram_tensor("ph", [128, _H], BF16, kind="ExternalInput")
    pht = nc.dram_tensor("pht", [_H, 128], BF16, kind="ExternalInput")
    ones2 = nc.dram_tensor("ones2", [128, 2], BF16, kind="ExternalInput")
    bdm = nc.dram_tensor("bdm", [64, 512], FP32, kind="ExternalInput")
    out = nc.dram_tensor("out", [TLOC, _D], BF16, kind="ExternalOutput")

    RG = [[0, 1], [2, 3], [4, 5], [6, 7]]

    with tile.TileContext(nc) as tc:
        with (
            tc.tile_pool(name="const", bufs=1) as cpool,
            tc.tile_pool(name="dram", bufs=1, space="DRAM") as dpool,
        ):
            xst_sb = cpool.tile([128, DC * TLOC], BF16)  # resident X^T
            qst_sb = cpool.tile([128, TLOC], BF16)       # softmax(Q)^T
            wk_sb = cpool.tile([128, _D], BF16)
            wq_sb = cpool.tile([128, _D], BF16)
            wv_sb = cpool.tile([128, DC * 512], BF16)
            op_sb = cpool.tile([128, 4 * _D], BF16)
            id_sb = cpool.tile([128, 128], BF16)
            ph_sb = cpool.tile([128, _H], BF16)
            pht_sb = cpool.tile([_H, 128], BF16)
            on_sb = cpool.tile([128, 2], BF16)
            ms_sb = cpool.tile([128, TLOC // 128], FP32)
            bdm_sb = cpool.tile([64, 512], FP32)
            kxp_sb = cpool.tile([128, 1032], FP32)   # packed KX | Ksum
            sh_sb = cpool.tile([64, 1032], FP32)     # reduced half state
            g_sb = cpool.tile([128, _D], BF16)       # gathered output weights

            rs_in = dpool.tile([128, 1032], FP32)
            rs_out = dpool.tile([64, 1032], FP32)
            ag_in = dpool.tile([64, _D], BF16)
            ag_out = dpool.tile([128, _D], BF16)
            wdum_in = dpool.tile([128, 2], BF16)
            wdum_out = dpool.tile([256, 2], BF16)

            # ------------- Phase A-K: KX / Ksum state accumulation ---------
            with (
                tc.tile_pool(name="xin", bufs=4) as xin,
                tc.tile_pool(name="esb", bufs=3) as esb,
                tc.tile_pool(name="e2", bufs=8) as e2p,
                tc.tile_pool(name="kt_ps", bufs=2, space="PSUM") as ktp,
                tc.tile_pool(name="scr_ps", bufs=2, space="PSUM") as scr,
                tc.tile_pool(name="kx_ps", bufs=1, space="PSUM") as kxp,
                tc.tile_pool(name="ks_ps", bufs=1, space="PSUM") as ksp,
            ):
                kx_ps = kxp.tile([128, _D], FP32)
                ks_ps = ksp.tile([128, 2], FP32)

                def ktx(k, xts, et):
                    """E-transpose + KX/KS accumulation for block k."""
                    e_ps = scr.tile([128, BT], BF16, tag="scr")
                    for i in range(NT):
                        nc.tensor.transpose(
                            e_ps[:, i * 128:(i + 1) * 128],
                            et[:, i * 128:(i + 1) * 128],
                            id_sb[:],
                        )
                    for i in range(NT):
                        e2 = e2p.tile([128, 128], BF16, tag="e2")
                        j = k * NT + i
                        nc.vector.tensor_scalar_mul(
                            e2[:], e_ps[:, i * 128:(i + 1) * 128],
                            ms_sb[:, j:j + 1],
                        )
                        first = (k == 0 and i == 0)
                        last = (k == NBLK - 1 and i == NT - 1)
                        nc.tensor.matmul(kx_ps[:, 0:512], e2[:],
                                         xts[i][:, 0:512],
                                         start=first, stop=last)
                        nc.tensor.matmul(kx_ps[:, 512:1024], e2[:],
                                         xts[i][:, 512:1024],
                                         start=first, stop=last)
                        nc.tensor.matmul(ks_ps[:], e2[:], on_sb[:],
                                         start=first, stop=last)

                prev = None
                for k in range(NBLK):
                    xb0 = k * DC * BT
                    if k == 0:
                        # critical-path loads on the sync ring: wk + the
                        # first X^T block gate the first matmul
                        nc.sync.dma_start(out=wk_sb[:], in_=wk.ap())
                        nc.sync.dma_start(
                            out=xst_sb[:, 0:DC * BT].rearrange(
                                "p (c t) -> p c t", c=DC),
                            in_=xst.ap().rearrange("(c p) t -> p c t", p=128)
                            [:, :, 0:BT])
                        # small constants ktx(0) needs, on gpsimd
                        nc.gpsimd.dma_start(out=id_sb[:], in_=ident.ap())
                        nc.gpsimd.dma_start(out=on_sb[:], in_=ones2.ap())
                        nc.gpsimd.dma_start(out=ms_sb[:], in_=ms.ap())
                        # tiny warm-up collective: absorbs the ncfw
                        # cold-start so the real RS later starts fast
                        nc.scalar.dma_start(out=wdum_in[:], in_=ones2.ap())
                        nc.gpsimd.collective_compute(
                            "AllGather",
                            mybir.AluOpType.bypass,
                            replica_groups=RG,
                            ins=[wdum_in.opt()],
                            outs=[wdum_out.opt()],
                        )
                    else:
                        # later X^T blocks stream on the gpsimd SWDGE ring,
                        # concurrent with the sync ring's xblk loads
                        nc.gpsimd.dma_start(
                            out=xst_sb[:, xb0:xb0 + DC * BT].rearrange(
                                "p (c t) -> p c t", c=DC),
                            in_=xst.ap().rearrange("(c p) t -> p c t", p=128)
                            [:, :, k * BT:(k + 1) * BT])
                    if k == 1:
                        nc.sync.dma_start(out=wq_sb[:], in_=wq.ap())

                    xblk = xin.tile([128, NT * _D], BF16, tag="xin")
                    nc.sync.dma_start(
                        out=xblk[:].rearrange("p (a d) -> p a d", a=NT),
                        in_=xs.ap()[k * BT:(k + 1) * BT, :].rearrange(
                            "(a p) d -> p a d", p=128))
                    xts = [xblk[:, i * _D:(i + 1) * _D] for i in range(NT)]

                    kt_ps = ktp.tile([128, BT], FP32, tag="kt")
                    for c in range(DC):
                        nc.tensor.matmul(
                            kt_ps[:], wk_sb[:, c * 128:(c + 1) * 128],
                            xst_sb[:, xb0 + c * BT:xb0 + (c + 1) * BT],
                            start=(c == 0), stop=(c == DC - 1),
                        )
                    et = esb.tile([128, BT], BF16, tag="et")
                    nc.scalar.activation(et[:], kt_ps[:], EXP)

                    if prev is not None:
                        ktx(*prev)
                    prev = (k, xts, et)
                ktx(*prev)

                # weights for phases B/C load after the X^T stream so they
                # don't delay the per-block critical path
                nc.gpsimd.dma_start(out=ph_sb[:], in_=ph.ap())
                nc.gpsimd.dma_start(out=pht_sb[:], in_=pht.ap())
                nc.gpsimd.dma_start(out=bdm_sb[:], in_=bdm.ap())
                for c2 in range(DC):
                    nc.gpsimd.dma_start(
                        out=wv_sb[:, c2 * 512:(c2 + 1) * 512],
                        in_=wv.ap()[:, c2 * 512:(c2 + 1) * 512])
                for c2 in range(4):
                    nc.gpsimd.dma_start(
                        out=op_sb[:, c2 * _D:(c2 + 1) * _D],
                        in_=op.ap()[:, c2 * _D:(c2 + 1) * _D])

                # pack state for the collective
                nc.vector.tensor_copy(kxp_sb[:, 0:512], kx_ps[:, 0:512])
                nc.scalar.copy(kxp_sb[:, 512:1024], kx_ps[:, 512:1024])
                nc.vector.tensor_copy(kxp_sb[:, 1024:1025], ks_ps[:, 0:1])
                nc.vector.memset(kxp_sb[:, 1025:1032], 0.0)

            # ---- ReduceScatter of the state (overlapped with A-Q) ---------
            nc.sync.dma_start(out=rs_in[:], in_=kxp_sb[:])
            nc.gpsimd.collective_compute(
                "ReduceScatter",
                mybir.AluOpType.add,
                replica_groups=RG,
                ins=[rs_in.opt()],
                outs=[rs_out.opt()],
            )
            nc.sync.dma_start(out=sh_sb[:], in_=rs_out[:])

            # ------------- Phase A-Q: softmax(Q)^T off resident X^T --------
            with (
                tc.tile_pool(name="eqsb", bufs=3) as eqsb,
                tc.tile_pool(name="srp", bufs=2) as srp,
                tc.tile_pool(name="qt_ps", bufs=2, space="PSUM") as qtp,
                tc.tile_pool(name="scr2_ps", bufs=2, space="PSUM") as scr2,
                tc.tile_pool(name="bsb", bufs=2) as bsb,
                tc.tile_pool(name="bsb1", bufs=1) as bsb1,
                tc.tile_pool(name="bps_small", bufs=2, space="PSUM") as bpss,
                tc.tile_pool(name="g_ps_pool", bufs=1, space="PSUM") as gpp,
            ):
                def qsoft(k, eq):
                    s_ps = scr2.tile([_H, BT], FP32, tag="scr")
                    nc.tensor.matmul(s_ps[:], ph_sb[:], eq[:], start=True,
                                     stop=True)
                    sr = srp.tile([_H, BT], FP32, tag="sr")
                    nc.vector.reciprocal_approx_fast(sr[:], s_ps[:])
                    srb = srp.tile([_H, BT], BF16, tag="srb")
                    nc.vector.tensor_copy(srb[:], sr[:])
                    bq_ps = scr2.tile([128, BT], FP32, tag="scr")
                    nc.tensor.matmul(bq_ps[:], pht_sb[:], srb[:], start=True,
                                     stop=True)
                    nc.vector.tensor_mul(
                        qst_sb[:, k * BT:(k + 1) * BT], eq[:], bq_ps[:]
                    )

                def qphase(blocks):
                    prev = None
                    for k in blocks:
                        qt_ps = qtp.tile([128, BT], FP32, tag="qt")
                        xb0 = k * DC * BT
                        for c in range(DC):
                            nc.tensor.matmul(
                                qt_ps[:], wq_sb[:, c * 128:(c + 1) * 128],
                                xst_sb[:, xb0 + c * BT:xb0 + (c + 1) * BT],
                                start=(c == 0), stop=(c == DC - 1),
                            )
                        eq = eqsb.tile([128, BT], BF16, tag="eq")
                        nc.scalar.activation(eq[:], qt_ps[:], EXP)
                        if prev is not None:
                            qsoft(*prev)
                        prev = (k, eq)
                    qsoft(*prev)

                qphase(range(NBLK - 2))   # blocks 0-5 cover the RS

                # ---- Phase B (half): G_half = bd(KXn@Wv) @ o_proj ---------
                # with the RS hidden under A-Q1, the PE flows straight from
                # A-Q1 into B with no idle gap, so HAM stays released
                rk = bsb1.tile([64, 1], FP32)
                nc.vector.reciprocal_approx_fast(rk[:], sh_sb[:, 1024:1025])
                kxn = bsb1.tile([128, _D], BF16)
                nc.vector.memset(kxn[64:128, :], 0.0)
                nc.vector.tensor_scalar_mul(kxn[0:64, :], sh_sb[:, 0:1024],
                                            rk[:])

                kxnt = []
                for c in range(DC):
                    tp = bpss.tile([128, 128], BF16, tag="bt")
                    nc.tensor.transpose(tp[:], kxn[:, c * 128:(c + 1) * 128],
                                        id_sb[:])
                    t_sb = bsb.tile([128, 128], BF16, tag="bts", bufs=8)
                    nc.vector.tensor_copy(t_sb[:], tp[:])
                    kxnt.append(t_sb)

                kv_ps = gpp.tile([64, 512], FP32, tag="kv")
                for c in range(DC):
                    nc.tensor.matmul(
                        kv_ps[:], kxnt[c][:, 0:64],
                        wv_sb[:, c * 512:(c + 1) * 512],
                        start=(c == 0), stop=(c == DC - 1))

                kvbd = bsb1.tile([128, 512], BF16)
                nc.vector.memset(kvbd[64:128, :], 0.0)
                nc.vector.tensor_mul(kvbd[0:64, :], kv_ps[:], bdm_sb[:])
                kvbdt = []
                for c in range(4):
                    tp = bpss.tile([128, 128], BF16, tag="bt")
                    nc.tensor.transpose(tp[:], kvbd[:, c * 128:(c + 1) * 128],
                                        id_sb[:])
                    t_sb = bsb.tile([128, 128], BF16, tag="btsf", bufs=4)
                    nc.vector.tensor_copy(t_sb[:], tp[:])
                    kvbdt.append(t_sb)

                # G in two sequential half-accumulations through one
                # 1-bank PSUM slot (the merged pool scope is bank-tight)
                g_half = bsb1.tile([64, _D], BF16)
                g_ps = gpp.tile([64, 512], FP32, tag="g")
                for r in range(4):
                    nc.tensor.matmul(
                        g_ps[:], kvbdt[r][:, 0:64],
                        op_sb[:, r * _D:r * _D + 512],
                        start=(r == 0), stop=(r == 3))
                nc.vector.tensor_copy(g_half[:, 0:512], g_ps[:])
                g_ps2 = gpp.tile([64, 512], FP32, tag="g")
                for r in range(4):
                    nc.tensor.matmul(
                        g_ps2[:], kvbdt[r][:, 0:64],
                        op_sb[:, r * _D + 512:(r + 1) * _D],
                        start=(r == 0), stop=(r == 3))
                nc.scalar.copy(g_half[:, 512:1024], g_ps2[:])

                # ---- AllGather the two G halves ---------------------------
                nc.sync.dma_start(out=ag_in[:], in_=g_half[:])
                nc.gpsimd.collective_compute(
                    "AllGather",
                    mybir.AluOpType.bypass,
                    replica_groups=RG,
                    ins=[ag_in.opt()],
                    outs=[ag_out.opt()],
                )
                nc.sync.dma_start(out=g_sb[:], in_=ag_out[:])

                # Q blocks 6-7 cover the AllGather latency
                qphase(range(NBLK - 2, NBLK))

            # ------------- Phase C: out = Qs @ G --------------------------
            with (
                tc.tile_pool(name="osb", bufs=3) as osb,
                tc.tile_pool(name="ops", bufs=3, space="PSUM") as ops,
            ):
                for j in range(TLOC // 256):
                    ot = osb.tile([128, 2 * _D], BF16, tag="osb")
                    for h2 in range(2):
                        i = 2 * j + h2
                        o_ps = ops.tile([128, _D], FP32, tag="ops")
                        lhs = qst_sb[:, i * 128:(i + 1) * 128]
                        nc.tensor.matmul(o_ps[:, 0:512], lhs, g_sb[:, 0:512],
                                         start=True, stop=True)
                        nc.tensor.matmul(o_ps[:, 512:1024], lhs,
                                         g_sb[:, 512:1024], start=True,
                                         stop=True)
                        dst = ot[:, h2 * _D:(h2 + 1) * _D]
                        if i % 2 == 0:
                            nc.vector.tensor_copy(dst, o_ps[:])
                        else:
                            nc.scalar.copy(dst, o_ps[:])
                    # alternate output rings so the stores are not FIFO
                    # serialized behind one engine
                    eng = nc.sync if j % 2 == 0 else nc.scalar
                    eng.dma_start(
                        out=out.ap()[j * 256:(j + 1) * 256, :].rearrange(
                            "(a p) d -> p a d", p=128),
                        in_=ot[:].rearrange("p (a d) -> p a d", a=2))

    nc.compile()
    return nc


def _host_inputs(X, attention_mask, Wk, Wq, Wv, o_proj):
    import ml_dtypes

    BF = ml_dtypes.bfloat16
    X = np.asarray(X, dtype=np.float32)
    mask = np.asarray(attention_mask, dtype=np.float32)
    Wk = np.asarray(Wk, dtype=np.float32)
    Wq = np.asarray(Wq, dtype=np.float32)
    Wv = np.asarray(Wv, dtype=np.float32)
    o_proj = np.asarray(o_proj, dtype=np.float32)

    wk_r = np.ascontiguousarray(
        Wk.reshape(DC, 128, _L).transpose(1, 0, 2).reshape(128, DC * _L)
    ).astype(BF)
    wq_r = np.ascontiguousarray(
        Wq.reshape(DC, 128, _L).transpose(1, 0, 2).reshape(128, DC * _L)
    ).astype(BF)
    # per head-half slices of Wv (columns) and o_proj (rows)
    wv_half = []
    op_half = []
    for half in range(2):
        wvh = Wv[:, half * 512:(half + 1) * 512]           # (1024, 512)
        wv_half.append(np.ascontiguousarray(
            wvh.reshape(DC, 128, 512).transpose(1, 0, 2).reshape(
                128, DC * 512)).astype(BF))
        oph = o_proj[half * 512:(half + 1) * 512, :]       # (512, 1024)
        op_half.append(np.ascontiguousarray(
            oph.reshape(4, 128, _D).transpose(1, 0, 2).reshape(
                128, 4 * _D)).astype(BF))
    ident = np.eye(128, dtype=BF)
    ph_m = np.zeros((128, _H), dtype=BF)
    for hh in range(_H):
        ph_m[hh * (_L // _H):(hh + 1) * (_L // _H), hh] = 1.0
    pht_m = np.ascontiguousarray(ph_m.T)
    ones2 = np.ones((128, 2), dtype=BF)
    # block-diag mask for 8 heads within a (64 x 512) half
    bdm_m = np.zeros((64, 512), dtype=np.float32)
    for hh in range(8):
        bdm_m[hh * (_L // _H):(hh + 1) * (_L // _H),
              hh * (_D // _H):(hh + 1) * (_D // _H)] = 1.0

    Xbf = X.astype(BF)
    in_maps = []
    for core in range(NCORES):
        b, half = core // 2, core % 2
        xsh = np.ascontiguousarray(Xbf[b, half * TLOC:(half + 1) * TLOC, :])
        xsth = np.ascontiguousarray(xsh.T)
        msh = np.ascontiguousarray(
            mask[b, half * TLOC:(half + 1) * TLOC]
            .reshape(TLOC // 128, 128).T)
        in_maps.append({
            "xs": xsh, "xst": xsth, "ms": msh, "wk": wk_r, "wq": wq_r,
            "wv": wv_half[half], "op": op_half[half], "ident": ident,
            "ph": ph_m, "pht": pht_m, "ones2": ones2, "bdm": bdm_m,
        })
    return in_maps


def _run(in_maps, trace=False):
    from concourse.bass_utils import run_bass_kernel_spmd

    if "nc" not in _cache:
        _cache["nc"] = _build()
    return run_bass_kernel_spmd(
        _cache["nc"], in_maps, list(range(NCORES)), trace=trace)


def kernel(X, attention_mask, Wk, Wq, Wv, o_proj, n_heads=16):
    in_maps = _host_inputs(X, attention_mask, Wk, Wq, Wv, o_proj)
    res = _run(in_maps)
    out = np.empty((_B, _T, _D), dtype=np.float32)
    for core in range(NCORES):
        b, half = core // 2, core % 2
        out[b, half * TLOC:(half + 1) * TLOC, :] = (
            res.results[core]["out"].astype(np.float32))
    return out


# revision 24
# speedup vs baseline: 1.1135x; 1.1135x over previous
"""BidLatte (linear-attention) Trainium2 kernel, 8-core SPMD.

Math (per batch b):
  K = X@Wk; Q = X@Wq; E = exp(K)*mask          (max-shift cancels exactly)
  Ksum = sum_t E;  KX = E^T @ X                (L x D state, avoids X@Wv)
  KXn = KX / Ksum; Kv = KXn @ Wv; Kv_bd = blockdiag_head(Kv)
  G = Kv_bd @ o_proj                           (o_proj folded into state)
  out = softmax_head(Q) @ G

Sharding: core 2i+j -> batch i, T-half j.

Schedule:
  A-K:   K-side state accumulation (DMA-bound).  X natural blocks stream
         on the sync HWDGE ring, X^T blocks on the gpsimd SWDGE ring
         (concurrent rings) into a resident SBUF X^T tile.  Q-side
         projections for blocks 0-2 are interleaved into the PE's
         DMA-starvation holes.  A tiny warm-up AllGather absorbs the
         ~11us ncfw cold-start.
  RS:    bf16 ReduceScatter of the packed (128 x 1032) state -- each
         core of a pair keeps the 8-head half it needs for phase B.
  A-Q:   head-softmax of blocks 0-2 + projection/softmax of blocks 3-5
         cover the RS.
  B:     transpose-free: the reduced half-state returns via an xbar
         DMA-transpose as KX^T chunks, Kv is computed transposed
         (lhsT = Wv 128-col slices), the blockdiag mask is applied in
         transposed form, and 1/Ksum is folded into the final G scale.
  A-Q2:  blocks 6-7 cover the small bf16 AllGather of G.
  C:     out = softmax(Q)^T^T @ G, stored bf16 (host converts to f32),
         stores alternating between the sync and gpsimd rings.

All matmuls bf16 (full PE rate); elementwise/exp/reductions and PSUM
accumulation fp32. Measured rel err vs fp32 reference ~6e-3.
"""
import numpy as np

_B, _T, _D, _L, _H = 4, 8192, 1024, 128, 16
NCORES = 8
TLOC = _T // 2  # tokens per core
BT = 512        # tokens per block
NBLK = TLOC // BT
NT = BT // 128  # t-tiles per block
DC = _D // 128  # d-chunks
NEARLY = 3      # Q blocks interleaved into A-K

_cache = {}


def _build():
    import concourse.bacc as bacc
    import concourse.mybir as mybir
    import concourse.tile as tile

    FP32 = mybir.dt.float32
    BF16 = mybir.dt.bfloat16
    EXP = mybir.ActivationFunctionType.Exp

    nc = bacc.Bacc("TRN2", target_bir_lowering=False, debug=False,
                   num_devices=NCORES)

    xs = nc.dram_tensor("xs", [TLOC, _D], BF16, kind="ExternalInput")
    xst = nc.dram_tensor("xst", [_D, TLOC], BF16, kind="ExternalInput")
    ms = nc.dram_tensor("ms", [128, TLOC // 128], FP32, kind="ExternalInput")
    wk = nc.dram_tensor("wk", [128, _D], BF16, kind="ExternalInput")
    wq = nc.dram_tensor("wq", [128, _D], BF16, kind="ExternalInput")
    wv = nc.dram_tensor("wv", [128, DC * 512], BF16, kind="ExternalInput")
    op = nc.dram_tensor("op", [128, 4 * _D], BF16, kind="ExternalInput")
    ident = nc.dram_tensor("ident", [128, 128], BF16, kind="ExternalInput")
    ph = nc.dram_tensor("ph", [128, _H], BF16, kind="ExternalInput")
    pht = nc.dram_tensor("pht", [_H, 128], BF16, kind="ExternalInput")
    ones2 = nc.dram_tensor("ones2", [128, 2], BF16, kind="ExternalInput")
    bdmt = nc.dram_tensor("bdmt", [128, 256], FP32, kind="ExternalInput")
    out = nc.dram_tensor("out", [TLOC, _D], BF16, kind="ExternalOutput")

    RG = [[0, 1], [2, 3], [4, 5], [6, 7]]

    with tile.TileContext(nc) as tc:
        with (
            tc.tile_pool(name="const", bufs=1) as cpool,
            tc.tile_pool(name="dram", bufs=1, space="DRAM") as dpool,
        ):
            xst_sb = cpool.tile([128, DC * TLOC], BF16)  # resident X^T
            qst_sb = cpool.tile([128, TLOC], BF16)       # softmax(Q)^T
            eqe_sb = cpool.tile([128, NEARLY * BT], BF16)  # early exp(Q)
            wk_sb = cpool.tile([128, _D], BF16)
            wq_sb = cpool.tile([128, _D], BF16)
            wv_sb = cpool.tile([128, DC * 512], BF16)
            op_sb = cpool.tile([128, 4 * _D], BF16)
            id_sb = cpool.tile([128, 128], BF16)
            ph_sb = cpool.tile([128, _H], BF16)
            pht_sb = cpool.tile([_H, 128], BF16)
            on_sb = cpool.tile([128, 2], BF16)
            ms_sb = cpool.tile([128, TLOC // 128], FP32)
            bdmt_sb = cpool.tile([128, 256], FP32)
            kxp_sb = cpool.tile([128, 1032], BF16)   # packed KX | Ksum
            sht_sb = cpool.tile([128, DC * 64], BF16)  # KX^T half chunks
            ksh_sb = cpool.tile([64, 8], BF16)       # Ksum half
            g_sb = cpool.tile([128, _D], BF16)       # gathered output weights

            rs_in = dpool.tile([128, 1032], BF16)
            rs_out = dpool.tile([64, 1032], BF16)
            ag_in = dpool.tile([64, _D], BF16)
            ag_out = dpool.tile([128, _D], BF16)
            wdum_in = dpool.tile([128, 2], BF16)
            wdum_out = dpool.tile([256, 2], BF16)

            # ------------- Phase A-K: KX / Ksum state accumulation ---------
            with (
                tc.tile_pool(name="xin", bufs=4) as xin,
                tc.tile_pool(name="esb", bufs=3) as esb,
                tc.tile_pool(name="e2", bufs=8) as e2p,
                tc.tile_pool(name="kt_ps", bufs=2, space="PSUM") as ktp,
                tc.tile_pool(name="scr_ps", bufs=2, space="PSUM") as scr,
                tc.tile_pool(name="kx_ps", bufs=1, space="PSUM") as kxp,
                tc.tile_pool(name="ks_ps", bufs=1, space="PSUM") as ksp,
            ):
                kx_ps = kxp.tile([128, _D], FP32)
                ks_ps = ksp.tile([128, 2], FP32)

                def ktx(k, xts, et):
                    """E-transpose + KX/KS accumulation for block k."""
                    e_ps = scr.tile([128, BT], BF16, tag="scr")
                    for i in range(NT):
                        nc.tensor.transpose(
                            e_ps[:, i * 128:(i + 1) * 128],
                            et[:, i * 128:(i + 1) * 128],
                            id_sb[:],
                        )
                    for i in range(NT):
                        e2 = e2p.tile([128, 128], BF16, tag="e2")
                        j = k * NT + i
                        nc.vector.tensor_scalar_mul(
                            e2[:], e_ps[:, i * 128:(i + 1) * 128],
                            ms_sb[:, j:j + 1],
                        )
                        first = (k == 0 and i == 0)
                        last = (k == NBLK - 1 and i == NT - 1)
                        nc.tensor.matmul(kx_ps[:, 0:512], e2[:],
                                         xts[i][:, 0:512],
                                         start=first, stop=last)
                        nc.tensor.matmul(kx_ps[:, 512:1024], e2[:],
                                         xts[i][:, 512:1024],
                                         start=first, stop=last)
                        nc.tensor.matmul(ks_ps[:], e2[:], on_sb[:],
                                         start=first, stop=last)

                prev = None
                for k in range(NBLK):
                    xb0 = k * DC * BT
                    if k == 0:
                        nc.sync.dma_start(out=wk_sb[:], in_=wk.ap())
                        nc.sync.dma_start(
                            out=xst_sb[:, 0:DC * BT].rearrange(
                                "p (c t) -> p c t", c=DC),
                            in_=xst.ap().rearrange("(c p) t -> p c t", p=128)
                            [:, :, 0:BT])
                        nc.gpsimd.dma_start(out=id_sb[:], in_=ident.ap())
                        nc.gpsimd.dma_start(out=on_sb[:], in_=ones2.ap())
                        nc.gpsimd.dma_start(out=ms_sb[:], in_=ms.ap())
                        # tiny warm-up collective: absorbs the ncfw
                        # cold-start so the real RS later starts fast
                        nc.scalar.dma_start(out=wdum_in[:], in_=ones2.ap())
                        nc.gpsimd.collective_compute(
                            "AllGather",
                            mybir.AluOpType.bypass,
                            replica_groups=RG,
                            ins=[wdum_in.opt()],
                            outs=[wdum_out.opt()],
                        )
                    else:
                        nc.gpsimd.dma_start(
                            out=xst_sb[:, xb0:xb0 + DC * BT].rearrange(
                                "p (c t) -> p c t", c=DC),
                            in_=xst.ap().rearrange("(c p) t -> p c t", p=128)
                            [:, :, k * BT:(k + 1) * BT])
                    if k == 1:
                        nc.sync.dma_start(out=wq_sb[:], in_=wq.ap())

                    xblk = xin.tile([128, NT * _D], BF16, tag="xin")
                    nc.sync.dma_start(
                        out=xblk[:].rearrange("p (a d) -> p a d", a=NT),
                        in_=xs.ap()[k * BT:(k + 1) * BT, :].rearrange(
                            "(a p) d -> p a d", p=128))
                    xts = [xblk[:, i * _D:(i + 1) * _D] for i in range(NT)]

                    kt_ps = ktp.tile([128, BT], FP32, tag="kt")
                    for c in range(DC):
                        nc.tensor.matmul(
                            kt_ps[:], wk_sb[:, c * 128:(c + 1) * 128],
                            xst_sb[:, xb0 + c * BT:xb0 + (c + 1) * BT],
                            start=(c == 0), stop=(c == DC - 1),
                        )
                    et = esb.tile([128, BT], BF16, tag="et")
                    nc.scalar.activation(et[:], kt_ps[:], EXP)

                    if prev is not None:
                        ktx(*prev)
                    prev = (k, xts, et)

                    # fill the PE's DMA-starvation holes with early Q-side
                    # projections off already-resident X^T blocks
                    if k in (2, 4, 6):
                        jq = (k - 2) // 2
                        jb0 = jq * DC * BT
                        qt_ps = ktp.tile([128, BT], FP32, tag="kt")
                        for c in range(DC):
                            nc.tensor.matmul(
                                qt_ps[:], wq_sb[:, c * 128:(c + 1) * 128],
                                xst_sb[:, jb0 + c * BT:jb0 + (c + 1) * BT],
                                start=(c == 0), stop=(c == DC - 1),
                            )
                        nc.scalar.activation(
                            eqe_sb[:, jq * BT:(jq + 1) * BT], qt_ps[:], EXP)
                ktx(*prev)

                # weights for phases B/C load after the X^T stream
                nc.gpsimd.dma_start(out=ph_sb[:], in_=ph.ap())
                nc.gpsimd.dma_start(out=pht_sb[:], in_=pht.ap())
                nc.gpsimd.dma_start(out=bdmt_sb[:], in_=bdmt.ap())
                for c2 in range(DC):
                    nc.gpsimd.dma_start(
                        out=wv_sb[:, c2 * 512:(c2 + 1) * 512],
                        in_=wv.ap()[:, c2 * 512:(c2 + 1) * 512])
                for c2 in range(4):
                    nc.gpsimd.dma_start(
                        out=op_sb[:, c2 * _D:(c2 + 1) * _D],
                        in_=op.ap()[:, c2 * _D:(c2 + 1) * _D])

                # pack state (bf16) for the collective
                nc.vector.tensor_copy(kxp_sb[:, 0:512], kx_ps[:, 0:512])
                nc.scalar.copy(kxp_sb[:, 512:1024], kx_ps[:, 512:1024])
                nc.vector.tensor_copy(kxp_sb[:, 1024:1025], ks_ps[:, 0:1])
                nc.vector.memset(kxp_sb[:, 1025:1032], 0.0)

            # ---- ReduceScatter of the state (overlapped with A-Q) ---------
            nc.sync.dma_start(out=rs_in[:], in_=kxp_sb[:])
            nc.gpsimd.collective_compute(
                "ReduceScatter",
                mybir.AluOpType.add,
                replica_groups=RG,
                ins=[rs_in.opt()],
                outs=[rs_out.opt()],
            )
            # reduced half state returns as KX^T chunks via xbar transpose
            nc.sync.dma_start_transpose(
                sht_sb[:].rearrange("p (c l) -> p c l", c=DC),
                rs_out[:, 0:1024])
            nc.sync.dma_start(out=ksh_sb[:], in_=rs_out[:, 1024:1032])

            # ------------- Phase A-Q + B + A-Q2 ---------------------------
            with (
                tc.tile_pool(name="eqsb", bufs=3) as eqsb,
                tc.tile_pool(name="srp", bufs=2) as srp,
                tc.tile_pool(name="bsb1", bufs=1) as bsb1,
                tc.tile_pool(name="qt_ps", bufs=2, space="PSUM") as qtp,
                tc.tile_pool(name="scr2_ps", bufs=3, space="PSUM") as scr2,
                tc.tile_pool(name="kvt_ps", bufs=1, space="PSUM") as kvtp,
                tc.tile_pool(name="g_ps_pool", bufs=1, space="PSUM") as gpp,
            ):
                def qsoft(k, eq):
                    s_ps = scr2.tile([_H, BT], FP32, tag="scr")
                    nc.tensor.matmul(s_ps[:], ph_sb[:], eq[:], start=True,
                                     stop=True)
                    sr = srp.tile([_H, BT], FP32, tag="sr")
                    nc.vector.reciprocal_approx_fast(sr[:], s_ps[:])
                    srb = srp.tile([_H, BT], BF16, tag="srb")
                    nc.vector.tensor_copy(srb[:], sr[:])
                    bq_ps = scr2.tile([128, BT], FP32, tag="scr")
                    nc.tensor.matmul(bq_ps[:], pht_sb[:], srb[:], start=True,
                                     stop=True)
                    nc.vector.tensor_mul(
                        qst_sb[:, k * BT:(k + 1) * BT], eq[:], bq_ps[:]
                    )

                def qphase(blocks):
                    prev = None
                    for k in blocks:
                        if k < NEARLY:
                            eq = eqe_sb[:, k * BT:(k + 1) * BT]
                        else:
                            qt_ps = qtp.tile([128, BT], FP32, tag="qt")
                            xb0 = k * DC * BT
                            for c in range(DC):
                                nc.tensor.matmul(
                                    qt_ps[:],
                                    wq_sb[:, c * 128:(c + 1) * 128],
                                    xst_sb[:, xb0 + c * BT:
                                           xb0 + (c + 1) * BT],
                                    start=(c == 0), stop=(c == DC - 1),
                                )
                            eqt = eqsb.tile([128, BT], BF16, tag="eq")
                            nc.scalar.activation(eqt[:], qt_ps[:], EXP)
                            eq = eqt[:]
                        if prev is not None:
                            qsoft(*prev)
                        prev = (k, eq)
                    qsoft(*prev)

                qphase(range(NBLK - 2))   # blocks 0-5 cover the RS

                # ---- Phase B (half, transpose-free) -----------------------
                ksf = bsb1.tile([64, 1], FP32)
                nc.vector.tensor_copy(ksf[:], ksh_sb[:, 0:1])
                rks = bsb1.tile([64, 1], FP32)
                nc.vector.reciprocal_approx_fast(rks[:], ksf[:])

                # Kv^T[d', l] accumulated directly from Wv slices x KX^T
                kvt_ps = kvtp.tile([128, 256], FP32)
                for e in range(4):
                    for c in range(DC):
                        nc.tensor.matmul(
                            kvt_ps[:, e * 64:(e + 1) * 64],
                            wv_sb[:, c * 512 + e * 128:
                                  c * 512 + (e + 1) * 128],
                            sht_sb[:, c * 64:(c + 1) * 64],
                            start=(c == 0), stop=(c == DC - 1),
                        )
                # blockdiag extract in transposed form
                kvbdt = bsb1.tile([128, 256], BF16)
                nc.vector.tensor_mul(kvbdt[:], kvt_ps[:], bdmt_sb[:])

                # G in two half-column accumulations; 1/Ksum folded into
                # the per-row scale of the output copy
                g_half = bsb1.tile([64, _D], BF16)
                g_ps = gpp.tile([64, 512], FP32, tag="g")
                for e in range(4):
                    nc.tensor.matmul(
                        g_ps[:], kvbdt[:, e * 64:(e + 1) * 64],
                        op_sb[:, e * _D:e * _D + 512],
                        start=(e == 0), stop=(e == 3))
                nc.vector.tensor_scalar_mul(g_half[:, 0:512], g_ps[:],
                                            rks[:])
                g_ps2 = gpp.tile([64, 512], FP32, tag="g")
                for e in range(4):
                    nc.tensor.matmul(
                        g_ps2[:], kvbdt[:, e * 64:(e + 1) * 64],
                        op_sb[:, e * _D + 512:(e + 1) * _D],
                        start=(e == 0), stop=(e == 3))
                nc.vector.tensor_scalar_mul(g_half[:, 512:1024], g_ps2[:],
                                            rks[:])

                # ---- AllGather the two G halves ---------------------------
                nc.sync.dma_start(out=ag_in[:], in_=g_half[:])
                nc.gpsimd.collective_compute(
                    "AllGather",
                    mybir.AluOpType.bypass,
                    replica_groups=RG,
                    ins=[ag_in.opt()],
                    outs=[ag_out.opt()],
                )
                nc.sync.dma_start(out=g_sb[:], in_=ag_out[:])

                # Q blocks 6-7 cover the AllGather latency
                qphase(range(NBLK - 2, NBLK))

            # ------------- Phase C: out = Qs @ G --------------------------
            with (
                tc.tile_pool(name="osb", bufs=3) as osb,
                tc.tile_pool(name="ops", bufs=3, space="PSUM") as ops,
            ):
                for j in range(TLOC // 256):
                    ot = osb.tile([128, 2 * _D], BF16, tag="osb")
                    for h2 in range(2):
                        i = 2 * j + h2
                        o_ps = ops.tile([128, _D], FP32, tag="ops")
                        lhs = qst_sb[:, i * 128:(i + 1) * 128]
                        nc.tensor.matmul(o_ps[:, 0:512], lhs, g_sb[:, 0:512],
                                         start=True, stop=True)
                        nc.tensor.matmul(o_ps[:, 512:1024], lhs,
                                         g_sb[:, 512:1024], start=True,
                                         stop=True)
                        dst = ot[:, h2 * _D:(h2 + 1) * _D]
                        if i % 2 == 0:
                            nc.vector.tensor_copy(dst, o_ps[:])
                        else:
                            nc.scalar.copy(dst, o_ps[:])
                    # alternate output rings (gpsimd is idle in phase C)
                    eng = nc.sync if j % 2 == 0 else nc.gpsimd
                    eng.dma_start(
                        out=out.ap()[j * 256:(j + 1) * 256, :].rearrange(
                            "(a p) d -> p a d", p=128),
                        in_=ot[:].rearrange("p (a d) -> p a d", a=2))

    nc.compile()
    return nc


def _host_inputs(X, attention_mask, Wk, Wq, Wv, o_proj):
    import ml_dtypes

    BF = ml_dtypes.bfloat16
    X = np.asarray(X, dtype=np.float32)
    mask = np.asarray(attention_mask, dtype=np.float32)
    Wk = np.asarray(Wk, dtype=np.float32)
    Wq = np.asarray(Wq, dtype=np.float32)
    Wv = np.asarray(Wv, dtype=np.float32)
    o_proj = np.asarray(o_proj, dtype=np.float32)

    wk_r = np.ascontiguousarray(
        Wk.reshape(DC, 128, _L).transpose(1, 0, 2).reshape(128, DC * _L)
    ).astype(BF)
    wq_r = np.ascontiguousarray(
        Wq.reshape(DC, 128, _L).transpose(1, 0, 2).reshape(128, DC * _L)
    ).astype(BF)
    # per head-half slices of Wv (columns) and o_proj (rows)
    wv_half = []
    op_half = []
    for half in range(2):
        wvh = Wv[:, half * 512:(half + 1) * 512]           # (1024, 512)
        wv_half.append(np.ascontiguousarray(
            wvh.reshape(DC, 128, 512).transpose(1, 0, 2).reshape(
                128, DC * 512)).astype(BF))
        oph = o_proj[half * 512:(half + 1) * 512, :]       # (512, 1024)
        op_half.append(np.ascontiguousarray(
            oph.reshape(4, 128, _D).transpose(1, 0, 2).reshape(
                128, 4 * _D)).astype(BF))
    ident = np.eye(128, dtype=BF)
    ph_m = np.zeros((128, _H), dtype=BF)
    for hh in range(_H):
        ph_m[hh * (_L // _H):(hh + 1) * (_L // _H), hh] = 1.0
    pht_m = np.ascontiguousarray(ph_m.T)
    ones2 = np.ones((128, 2), dtype=BF)
    # transposed block-diag mask: bdmt[p, e*64+l] = 1 iff local d'-row
    # (e*128+p) belongs to the head of local column l
    bdmt_m = np.zeros((128, 256), dtype=np.float32)
    for e in range(4):
        for p in range(128):
            dloc = e * 128 + p
            hh = dloc // 64
            bdmt_m[p, e * 64 + hh * 8:e * 64 + (hh + 1) * 8] = 1.0

    Xbf = X.astype(BF)
    in_maps = []
    for core in range(NCORES):
        b, half = core // 2, core % 2
        xsh = np.ascontiguousarray(Xbf[b, half * TLOC:(half + 1) * TLOC, :])
        xsth = np.ascontiguousarray(xsh.T)
        msh = np.ascontiguousarray(
            mask[b, half * TLOC:(half + 1) * TLOC]
            .reshape(TLOC // 128, 128).T)
        in_maps.append({
            "xs": xsh, "xst": xsth, "ms": msh, "wk": wk_r, "wq": wq_r,
            "wv": wv_half[half], "op": op_half[half], "ident": ident,
            "ph": ph_m, "pht": pht_m, "ones2": ones2, "bdmt": bdmt_m,
        })
    return in_maps


def _run(in_maps, trace=False):
    from concourse.bass_utils import run_bass_kernel_spmd

    if "nc" not in _cache:
        _cache["nc"] = _build()
    return run_bass_kernel_spmd(
        _cache["nc"], in_maps, list(range(NCORES)), trace=trace)


def kernel(X, attention_mask, Wk, Wq, Wv, o_proj, n_heads=16):
    in_maps = _host_inputs(X, attention_mask, Wk, Wq, Wv, o_proj)
    res = _run(in_maps)
    out = np.empty((_B, _T, _D), dtype=np.float32)
    for core in range(NCORES):
        b, half = core // 2, core % 2
        out[b, half * TLOC:(half + 1) * TLOC, :] = (
            res.results[core]["out"].astype(np.float32))
    return out
